# revision 47
# baseline (speedup 1.0000x reference)
"""GQA attention prefill (B=2, S=1024, D=4096, H=32, KVH=8, HD=128) on 8 TRN2
NeuronCores.

Sharding: tensor-parallel over heads. Core c owns KV head c and query heads
4c..4c+3 (GQA groups align with cores), i.e. column-shards of wq/wk/wv and the
matching row-shard of wo. Each core computes its partial `attn_c @ wo_c`
([B*S, D]); the host sums the 8 partials.

Device layouts (per core):
  xT   [D, B*S]   bf16  host-transposed activations (contraction dim on partitions)
  wq   [D, 512]   bf16  rope-permuted (even dims then odd dims within each head)
  wk   [D, 128]   bf16  rope-permuted
  wv   [D, 128]   bf16
  wo   [512, D]   bf16
  c2   [128, S]   f32   rope cos table, duplicated across the two 64-row halves
  s2   [128, S]   f32   rope sin table, [-sin; +sin]
  mt   [128,128]  f32   causal diagonal-block additive mask * sqrt(HD)   (causal)
  mt   [S, S]     bf16  full transposed additive mask * sqrt(HD)         (general)

Q/K are produced transposed ([d, tok]) straight out of the projection matmul;
scores are computed transposed ([k, q]) so softmax denominators come from a
ones-vector matmul and P^T feeds the PV matmul with no transposes anywhere.
Rope's even/odd pairing is turned into a contiguous half-swap by permuting the
weight columns; the swap itself is two SBUF->SBUF partition-block DMAs.
Softmax skips the max-subtraction (scores are O(10); exp accumulates in fp32).
"""

import math
from contextlib import ExitStack

import numpy as np
import ml_dtypes

import concourse.bass as bass
import concourse.mybir as mybir
import concourse.tile as tile
from concourse import bacc
from concourse.bass_utils import run_bass_kernel_spmd

BF16 = mybir.dt.bfloat16
F32 = mybir.dt.float32
NPBF16 = ml_dtypes.bfloat16

B, S, D, H, KVH, HD = 2, 1024, 4096, 32, 8, 128
NCORES = 8
NH = H // NCORES          # 4 query heads per core
DC = D // 128             # 32 contraction chunks
TB = 256                  # token chunk for the QKV projection
SQ = 1.0 / math.sqrt(HD)


def _chunks(q0, qend, step=512):
    qs = q0
    while qs < qend:
        nq = min(step, qend - qs)
        yield qs, nq
        qs += nq


def build_program(causal, s=S, d=D, tb=TB):
    """Build the per-core SPMD program. s/d/tb are overridable for sim tests."""
    dc = d // 128
    nkb = s // 128            # number of 128-wide key/query blocks per batch
    ntc = s // tb             # token chunks per batch
    qcols = NH * HD

    # pT packing offsets: causal keeps only k-block ki's valid q range [128ki, s)
    if causal:
        q0s = [ki * 128 for ki in range(nkb)]
    else:
        q0s = [0] * nkb
    offs, acc = [], 0
    for ki in range(nkb):
        offs.append(acc)
        acc += s - q0s[ki]
    pt_len = acc

    nc = bacc.Bacc(
        "TRN2",
        target_bir_lowering=False,
        debug=False,
        enable_asserts=False,
        num_devices=1,
    )
    # all inputs are host-pretiled into the exact SBUF layouts, so every DMA
    # below reads fully contiguous per-partition lines
    xT = nc.dram_tensor("xT", [128, B * ntc, dc, tb], BF16, kind="ExternalInput").ap()
    wq = nc.dram_tensor("wq", [128, NH, dc, HD], BF16, kind="ExternalInput").ap()
    wk = nc.dram_tensor("wk", [128, dc, HD], BF16, kind="ExternalInput").ap()
    wv = nc.dram_tensor("wv", [128, dc, HD], BF16, kind="ExternalInput").ap()
    wo = nc.dram_tensor("wo", [128, d // 512, NH, 512], BF16, kind="ExternalInput").ap()
    sw = nc.dram_tensor("sw", [128, 128], BF16, kind="ExternalInput").ap()
    c2 = nc.dram_tensor("c2", [128, s], F32, kind="ExternalInput").ap()
    s2 = nc.dram_tensor("s2", [128, s], F32, kind="ExternalInput").ap()
    if causal:
        mt = nc.dram_tensor("mt", [128, 128], F32, kind="ExternalInput").ap()
    else:
        mt = nc.dram_tensor("mt", [s, s], BF16, kind="ExternalInput").ap()
    out = nc.dram_tensor("out", [B * s, d], BF16, kind="ExternalOutput").ap()

    with tile.TileContext(nc) as tc:
        with ExitStack() as ctx:
            const = ctx.enter_context(tc.tile_pool(name="const", bufs=1))
            xpool = ctx.enter_context(tc.tile_pool(name="xpool", bufs=2))
            wopool = ctx.enter_context(tc.tile_pool(name="wopool", bufs=2))
            qkv = ctx.enter_context(tc.tile_pool(name="qkv", bufs=2))
            ptp = ctx.enter_context(tc.tile_pool(name="ptp", bufs=2))
            rp = ctx.enter_context(tc.tile_pool(name="rp", bufs=3))
            small = ctx.enter_context(tc.tile_pool(name="small", bufs=2))
            oev = ctx.enter_context(tc.tile_pool(name="oev", bufs=2))
            psm = ctx.enter_context(tc.tile_pool(name="psm", bufs=4, space="PSUM"))
            pss = ctx.enter_context(tc.tile_pool(name="pss", bufs=2, space="PSUM"))

            # resident constants / weights — spread the startup loads over
            # several engines' DMA queues so they transfer in parallel and
            # the first projection matmuls can start early
            # queue order matters: each queue drains in emission order, so
            # load what the first projections need (wk, wq0..) ahead of the
            # rope/exp tables used a few microseconds later
            wk_sb = const.tile([128, dc, HD], BF16)
            nc.gpsimd.dma_start(wk_sb[:], wk[:])
            wq_sb = const.tile([128, NH, dc, HD], BF16)
            nc.scalar.dma_start(wq_sb[:, 0, :, :], wq[:, 0, :, :])
            wv_sb = const.tile([128, dc, HD], BF16)
            nc.gpsimd.dma_start(wv_sb[:], wv[:])
            nc.gpsimd.dma_start(wq_sb[:, 1, :, :], wq[:, 1, :, :])
            nc.scalar.dma_start(wq_sb[:, 2, :, :], wq[:, 2, :, :])
            nc.gpsimd.dma_start(wq_sb[:, 3, :, :], wq[:, 3, :, :])
            c2_sb = const.tile([128, s], F32)
            nc.scalar.dma_start(c2_sb[:], c2[:])
            s2_sb = const.tile([128, s], F32)
            nc.scalar.dma_start(s2_sb[:], s2[:])
            if causal:
                mt_sb = const.tile([128, 128], F32)
                nc.scalar.dma_start(mt_sb[:], mt[:])
            else:
                mt_sb = const.tile([128, nkb, s], BF16)
                nc.scalar.dma_start(mt_sb[:], mt.rearrange("(kb p) q -> p kb q", p=128))
            ones_sb = const.tile([128, 1], BF16)
            nc.vector.memset(ones_sb[:], 1.0)
            id_sb = const.tile([128, 128], BF16)
            nc.gpsimd.dma_start(id_sb[:], sw[:])

            def rope(ps, tok0, w, out_slice):
                """ps: [128, w] psum with raw projected Q/K block (d-permuted).
                out = raw*c2 + halfswap(raw)*s2, written as bf16 to out_slice.
                Only the ACT eviction touches PSUM; the swap is two SBUF
                partition-block DMAs and the muls run from SBUF on gpsimd/DVE."""
                raw = rp.tile([128, w], BF16, tag="raw", name=f"raw_{tok0}")
                nc.scalar.copy(raw[:], ps[:, :w])
                # swap posted from the ACT engine that just wrote raw: no
                # cross-engine wait, and ACT's DMA queue has no big transfers
                swt = rp.tile([128, w], BF16, tag="swt", name=f"swt_{tok0}")
                nc.scalar.dma_start(swt[0:64, :], raw[64:128, :])
                nc.scalar.dma_start(swt[64:128, :], raw[0:64, :])
                t1 = rp.tile([128, w], F32, tag="t1", name=f"t1_{tok0}")
                nc.vector.tensor_mul(t1[:], swt[:], s2_sb[:, tok0 : tok0 + w])
                t2 = rp.tile([128, w], F32, tag="t2", name=f"t2_{tok0}")
                nc.vector.tensor_mul(t2[:], raw[:], c2_sb[:, tok0 : tok0 + w])
                nc.gpsimd.tensor_add(out_slice, t2[:], t1[:])

            def phase2(b):
                """Stream xT, project Q/K/V for batch b. Returns the
                per-batch activation tiles."""
                qT_sb = qkv.tile([128, NH, s], BF16, tag="qT", name=f"qT_{b}")
                kT_sb = qkv.tile([128, s], BF16, tag="kT", name=f"kT_{b}")
                vT_sb = qkv.tile([128, s], BF16, tag="vT", name=f"vT_{b}")
                v_sb = qkv.tile([128, nkb, HD], BF16, tag="v", name=f"v_{b}")
                attnT_sb = qkv.tile([128, NH, s], BF16, tag="attnT", name=f"attnT_{b}")

                # evictions/rope are emitted one projection late, so each
                # engine's FIFO only sees work whose PSUM inputs are (nearly)
                # ready — avoids head-of-line blocking behind matmul chains.
                pending = []

                def flush(keep):
                    while len(pending) > keep:
                        kind, ps, tok0_ = pending.pop(0)
                        if kind == "k":
                            rope(ps, tok0_, tb, kT_sb[:, tok0_ : tok0_ + tb])
                        elif kind.startswith("q"):
                            h = int(kind[1:])
                            rope(ps, tok0_, tb, qT_sb[:, h, tok0_ : tok0_ + tb])
                        else:  # vt
                            nc.vector.tensor_copy(vT_sb[:, tok0_ : tok0_ + tb], ps[:])
                            for m2 in range(tb // 128):
                                kb = tok0_ // 128 + m2
                                vtp = pss.tile(
                                    [128, HD], BF16, tag="sm", name=f"vtp_{b}_{kb}"
                                )
                                nc.tensor.transpose(
                                    vtp[:], vT_sb[:, kb * 128 : (kb + 1) * 128], id_sb[:]
                                )
                                nc.scalar.copy(v_sb[:, kb, :], vtp[:])

                for t4 in range(ntc):
                    tok0 = t4 * tb
                    xc = xpool.tile([128, dc, tb], BF16, tag="xc", name=f"xc_{b}_{t4}")
                    nc.sync.dma_start(xc[:], xT[:, b * ntc + t4, :, :])
                    # K projection -> kT (transposed layout, rope applied)
                    k_ps = psm.tile([128, tb], F32, tag="mm", name=f"kps_{b}_{t4}")
                    for c in range(dc):
                        nc.tensor.matmul(
                            k_ps[:],
                            wk_sb[:, c, :],
                            xc[:, c, :],
                            start=(c == 0),
                            stop=(c == dc - 1),
                        )
                    pending.append(("k", k_ps, tok0))
                    flush(1)
                    # V projection, transposed like K (wide-N matmuls), then
                    # PE-transposed back to the natural [tok, d] layout
                    vt_ps = psm.tile([128, tb], F32, tag="mm", name=f"vtps_{b}_{t4}")
                    for c in range(dc):
                        nc.tensor.matmul(
                            vt_ps[:],
                            wv_sb[:, c, :],
                            xc[:, c, :],
                            start=(c == 0),
                            stop=(c == dc - 1),
                        )
                    pending.append(("vt", vt_ps, tok0))
                    flush(1)
                    # Q projection -> qT (transposed layout, rope applied)
                    for h in range(NH):
                        q_ps = psm.tile([128, tb], F32, tag="mm", name=f"qps_{b}_{t4}_{h}")
                        for c in range(dc):
                            nc.tensor.matmul(
                                q_ps[:],
                                wq_sb[:, h, c, :],
                                xc[:, c, :],
                                start=(c == 0),
                                stop=(c == dc - 1),
                            )
                        pending.append((f"q{h}", q_ps, tok0))
                        flush(1)
                flush(0)
                return dict(qT=qT_sb, kT=kT_sb, vT=vT_sb, v=v_sb, attnT=attnT_sb)

            def attn(b, T):
                """Attention for batch b, software-pipelined: PV of head h-1
                is emitted after the scores/sums of head h, so the softmax-
                denominator chain of head h-1 hides under head h's PE work."""
                qT_sb, kT_sb, v_sb, attnT_sb = T["qT"], T["kT"], T["v"], T["attnT"]
                stage1 = {}

                def attn_scores(h):
                    pT = ptp.tile([128, pt_len], BF16, tag="pt", name=f"pt_{b}_{h}")
                    sums = pss.tile([1, s], F32, tag="sums", bufs=1, name=f"sums_{b}_{h}")
                    for ki in range(nkb):
                        q0 = q0s[ki]
                        # diagonal-containing chunk (the first) goes last, so
                        # its extra DVE mask-add overlaps the other chunk's
                        # mask-free matmul->exp flow
                        for qs_, nq in reversed(list(_chunks(q0, s))):
                            sc = psm.tile([128, 512], F32, tag="mm", name=f"sc_{b}_{h}_{ki}_{qs_}")
                            nc.tensor.matmul(
                                sc[:, :nq],
                                kT_sb[:, ki * 128 : (ki + 1) * 128],
                                qT_sb[:, h, qs_ : qs_ + nq],
                                start=True,
                                stop=True,
                            )
                            if causal:
                                if qs_ == q0:  # diagonal block
                                    nc.vector.tensor_add(
                                        sc[:, 0:128], sc[:, 0:128], mt_sb[:]
                                    )
                            else:
                                nc.vector.tensor_add(
                                    sc[:, :nq], sc[:, :nq], mt_sb[:, ki, qs_ : qs_ + nq]
                                )
                            po = offs[ki] + qs_ - q0
                            nc.scalar.activation(
                                pT[:, po : po + nq],
                                sc[:, :nq],
                                mybir.ActivationFunctionType.Exp,
                                scale=SQ,
                            )
                            # denominators accumulate in PSUM across ki; the
                            # causal q-ranges nest, so ki==0 (full range)
                            # starts the group for every column.
                            nc.tensor.matmul(
                                sums[0:1, qs_ : qs_ + nq],
                                ones_sb[:],
                                pT[:, po : po + nq],
                                start=(ki == 0),
                                stop=(ki == nkb - 1),
                                skip_group_check=True,
                            )
                    # denominator chain, split into <=512 column pieces so each
                    # serial stage is short and pieces pipeline across engines
                    nhalf = (s + 511) // 512
                    width = s // nhalf
                    rbrs = []
                    for hs in range(nhalf):
                        ssb = small.tile([1, width], F32, tag="ssb", bufs=4, name=f"ssb_{b}_{h}_{hs}")
                        nc.scalar.copy(ssb[0:1, :], sums[0:1, hs * width : (hs + 1) * width])
                        rb = small.tile([128, width], F32, tag="rb", bufs=4, name=f"rb_{b}_{h}_{hs}")
                        nc.gpsimd.partition_broadcast(rb[:], ssb[0:1, :])
                        rbr = small.tile([128, width], F32, tag="rbr", bufs=4, name=f"rbr_{b}_{h}_{hs}")
                        nc.vector.reciprocal_approx_fast(rbr[:], rb[:])
                        rbrs.append(rbr)
                    return pT, rbrs, width

                def attn_pv(h):
                    # wide-N PV: per q-chunk, each k-block contributes one
                    # matmul over its (nested) valid q range, accumulating in
                    # PSUM — ki==0 always covers the whole chunk, so it opens
                    # the group for every column (same trick as the sums).
                    pT, rbrs, width = stage1.pop(h)
                    for ci, (qs0, w) in enumerate(_chunks(0, s)):
                        o_ps = psm.tile([128, 512], F32, tag="mm", name=f"ops_{b}_{h}_{ci}")
                        kis = [
                            k for k in range(nkb) if (not causal) or q0s[k] < qs0 + w
                        ]
                        for j, ki in enumerate(kis):
                            qlo = max(q0s[ki], qs0)
                            nc.tensor.matmul(
                                o_ps[:, qlo - qs0 : w],
                                v_sb[:, ki, :],
                                pT[:, offs[ki] + qlo - q0s[ki] : offs[ki] + qs0 + w - q0s[ki]],
                                start=(j == 0),
                                stop=(j == len(kis) - 1),
                                skip_group_check=True,
                            )
                        nc.vector.tensor_mul(
                            attnT_sb[:, h, qs0 : qs0 + w],
                            o_ps[:, :w],
                            rbrs[qs0 // width][:, qs0 % width : qs0 % width + w],
                        )

                for h in range(NH):
                    stage1[h] = attn_scores(h)
                    if h > 0:
                        attn_pv(h - 1)
                attn_pv(NH - 1)

            def oproj(b, T):
                """Output projection (partial over this core's wo rows)."""
                attnT_sb = T["attnT"]
                for nb in range(d // 512):
                    wo_nb = wopool.tile([128, NH, 512], BF16, tag="wo", name=f"wo_{b}_{nb}")
                    nc.sync.dma_start(wo_nb[:], wo[:, nb, :, :])
                    for tp in range(nkb // 2):
                        ot = oev.tile([128, 2, 512], BF16, tag="ot", bufs=4, name=f"ot_{b}_{nb}_{tp}")
                        for half in range(2):
                            tbk = tp * 2 + half
                            o2 = psm.tile([128, 512], F32, tag="mm", name=f"o2_{b}_{nb}_{tbk}")
                            for h in range(NH):
                                nc.tensor.matmul(
                                    o2[:],
                                    attnT_sb[:, h, tbk * 128 : (tbk + 1) * 128],
                                    wo_nb[:, h, :],
                                    start=(h == 0),
                                    stop=(h == NH - 1),
                                )
                            if half == 0:
                                nc.scalar.copy(ot[:, half, :], o2[:])
                            else:
                                nc.vector.tensor_copy(ot[:, half, :], o2[:])
                        (nc.sync if tp % 2 == 0 else nc.gpsimd).dma_start(
                            out[
                                b * s + tp * 256 : b * s + (tp + 1) * 256,
                                nb * 512 : (nb + 1) * 512,
                            ].rearrange("(rh p) n -> p rh n", p=128),
                            ot[:],
                        )

            # phase order: batch b+1's projections are emitted before batch
            # b's output projection, so b's attention tail (softmax chains)
            # hides under b+1's dense matmul stream.
            T0 = phase2(0)
            attn(0, T0)
            if B > 1:
                T1 = phase2(1)
                oproj(0, T0)
                attn(1, T1)
                oproj(1, T1)
            else:
                oproj(0, T0)
    nc.compile()
    return nc


# ---------------------------------------------------------------------------
# host side
# ---------------------------------------------------------------------------

_PERM = np.concatenate([np.arange(0, HD, 2), np.arange(1, HD, 2)])
_CACHE = {}


def _tile_x(x, s=S, d=D, tb=TB):
    """[B, s, d] f32 -> [128, B*ntc, dc, tb] bf16 (SBUF layout of xT chunks)."""
    ntc, dc = s // tb, d // 128
    t = x.reshape(B, ntc, tb, dc, 128).transpose(4, 0, 1, 3, 2)
    return np.ascontiguousarray(t.reshape(128, B * ntc, dc, tb)).astype(NPBF16)


def _tile_wq(w, d=D):
    """[d, NH*HD] f32 (already rope-permuted) -> [128, NH, dc, HD] bf16."""
    dc = d // 128
    t = w.reshape(dc, 128, NH, HD).transpose(1, 2, 0, 3)
    return np.ascontiguousarray(t).astype(NPBF16)


def _tile_wkv(w, d=D):
    """[d, HD] f32 -> [128, dc, HD] bf16."""
    dc = d // 128
    return np.ascontiguousarray(w.reshape(dc, 128, HD).transpose(1, 0, 2)).astype(NPBF16)


def _tile_wo(w, d=D):
    """[NH*HD, d] f32 -> [128, d//512, NH, 512] bf16."""
    t = w.reshape(NH, 128, d // 512, 512).transpose(1, 2, 0, 3)
    return np.ascontiguousarray(t).astype(NPBF16)


def _get_program(causal):
    if causal not in _CACHE:
        _CACHE[causal] = build_program(causal)
    return _CACHE[causal]


def _is_causal(mask):
    iu = np.triu_indices(S, 1)
    il = np.tril_indices(S)
    return bool(np.all(mask[il] == 0.0) and np.all(mask[iu] < -1e8))


def make_in_maps(x, cos, sin, mask, wq, wk, wv, wo, causal):
    x = np.asarray(x, dtype=np.float32)
    cos = np.asarray(cos, dtype=np.float32)
    sin = np.asarray(sin, dtype=np.float32)
    mask = np.asarray(mask, dtype=np.float32)
    wq = np.asarray(wq, dtype=np.float32)
    wk = np.asarray(wk, dtype=np.float32)
    wv = np.asarray(wv, dtype=np.float32)
    wo = np.asarray(wo, dtype=np.float32)

    xT = _tile_x(x)
    c2 = np.ascontiguousarray(np.concatenate([cos.T, cos.T], 0)).astype(np.float32)
    s2 = np.ascontiguousarray(np.concatenate([-sin.T, sin.T], 0)).astype(np.float32)
    swm = np.eye(128, dtype=np.float32).astype(NPBF16)  # transpose identity
    if causal:
        mt = np.ascontiguousarray(mask[:128, :128].T * math.sqrt(HD)).astype(np.float32)
    else:
        mt = np.ascontiguousarray(mask.T * math.sqrt(HD)).astype(NPBF16)

    in_maps = []
    for c in range(NCORES):
        wq_c = wq[:, c * NH * HD : (c + 1) * NH * HD].reshape(D, NH, HD)[:, :, _PERM]
        wq_c = _tile_wq(wq_c.reshape(D, NH * HD))
        wk_c = _tile_wkv(np.ascontiguousarray(wk[:, c * HD : (c + 1) * HD][:, _PERM]))
        wv_c = _tile_wkv(np.ascontiguousarray(wv[:, c * HD : (c + 1) * HD]))
        wo_c = _tile_wo(np.ascontiguousarray(wo[c * NH * HD : (c + 1) * NH * HD, :]))
        in_maps.append(
            {
                "xT": xT,
                "wq": wq_c,
                "wk": wk_c,
                "wv": wv_c,
                "wo": wo_c,
                "sw": swm,
                "c2": c2,
                "s2": s2,
                "mt": mt,
            }
        )
    return in_maps


def run(in_maps, causal, **kwargs):
    nc = _get_program(causal)
    return run_bass_kernel_spmd(nc, in_maps, core_ids=list(range(NCORES)), **kwargs)


def kernel(x, start_pos, cos, sin, mask, wq, wk, wv, wo):
    mask = np.asarray(mask, dtype=np.float32)
    causal = _is_causal(mask)
    in_maps = make_in_maps(x, cos, sin, mask, wq, wk, wv, wo, causal)
    res = run(in_maps, causal)
    acc = np.zeros((B * S, D), dtype=np.float32)
    for c in range(NCORES):
        acc += np.asarray(res.results[c]["out"], dtype=np.float32)
    return acc.reshape(B, S, D)


# revision 52
# speedup vs baseline: 1.0698x; 1.0698x over previous
"""GQA attention prefill (B=2, S=1024, D=4096, H=32, KVH=8, HD=128) on 8 TRN2
NeuronCores.

Sharding: tensor-parallel over heads. Core c owns KV head c and query heads
4c..4c+3 (GQA groups align with cores), i.e. column-shards of wq/wk/wv and the
matching row-shard of wo. Each core computes its partial `attn_c @ wo_c`
([B*S, D]); the host sums the 8 partials.

Device layouts (per core):
  xT   [D, B*S]   bf16  host-transposed activations (contraction dim on partitions)
  wq   [D, 512]   bf16  rope-permuted (even dims then odd dims within each head)
  wk   [D, 128]   bf16  rope-permuted
  wv   [D, 128]   bf16
  wo   [512, D]   bf16
  c2   [128, S]   f32   rope cos table, duplicated across the two 64-row halves
  s2   [128, S]   f32   rope sin table, [-sin; +sin]
  mt   [128,128]  f32   causal diagonal-block additive mask * sqrt(HD)   (causal)
  mt   [S, S]     bf16  full transposed additive mask * sqrt(HD)         (general)

Q/K are produced transposed ([d, tok]) straight out of the projection matmul;
scores are computed transposed ([k, q]) so softmax denominators come from a
ones-vector matmul and P^T feeds the PV matmul with no transposes anywhere.
Rope's even/odd pairing is turned into a contiguous half-swap by permuting the
weight columns; the swap itself is two SBUF->SBUF partition-block DMAs.
Softmax skips the max-subtraction (scores are O(10); exp accumulates in fp32).
"""

import math
from contextlib import ExitStack

import numpy as np
import ml_dtypes

import concourse.bass as bass
import concourse.mybir as mybir
import concourse.tile as tile
from concourse import bacc
from concourse.bass_utils import run_bass_kernel_spmd

BF16 = mybir.dt.bfloat16
F32 = mybir.dt.float32
NPBF16 = ml_dtypes.bfloat16

B, S, D, H, KVH, HD = 2, 1024, 4096, 32, 8, 128
NCORES = 8
NH = H // NCORES          # 4 query heads per core
DC = D // 128             # 32 contraction chunks
TB = 256                  # token chunk for the QKV projection
SQ = 1.0 / math.sqrt(HD)


def _chunks(q0, qend, step=512):
    qs = q0
    while qs < qend:
        nq = min(step, qend - qs)
        yield qs, nq
        qs += nq


def build_program(causal, s=S, d=D, tb=TB):
    """Build the per-core SPMD program. s/d/tb are overridable for sim tests."""
    dc = d // 128
    nkb = s // 128            # number of 128-wide key/query blocks per batch
    ntc = s // tb             # token chunks per batch
    qcols = NH * HD

    # pT packing offsets: causal keeps only k-block ki's valid q range [128ki, s)
    if causal:
        q0s = [ki * 128 for ki in range(nkb)]
    else:
        q0s = [0] * nkb
    offs, acc = [], 0
    for ki in range(nkb):
        offs.append(acc)
        acc += s - q0s[ki]
    pt_len = acc

    nc = bacc.Bacc(
        "TRN2",
        target_bir_lowering=False,
        debug=False,
        enable_asserts=False,
        num_devices=1,
    )
    # all inputs are host-pretiled into the exact SBUF layouts, so every DMA
    # below reads fully contiguous per-partition lines
    xT = nc.dram_tensor("xT", [128, B * ntc, dc, tb], BF16, kind="ExternalInput").ap()
    wq = nc.dram_tensor("wq", [128, NH, dc, HD], BF16, kind="ExternalInput").ap()
    wk = nc.dram_tensor("wk", [128, dc, HD], BF16, kind="ExternalInput").ap()
    wv = nc.dram_tensor("wv", [128, dc, HD], BF16, kind="ExternalInput").ap()
    wo = nc.dram_tensor("wo", [128, d // 512, NH, 512], BF16, kind="ExternalInput").ap()
    sw = nc.dram_tensor("sw", [128, 128], BF16, kind="ExternalInput").ap()
    c2 = nc.dram_tensor("c2", [128, s], F32, kind="ExternalInput").ap()
    s2 = nc.dram_tensor("s2", [128, s], F32, kind="ExternalInput").ap()
    if causal:
        mt = nc.dram_tensor("mt", [128, 128], F32, kind="ExternalInput").ap()
    else:
        mt = nc.dram_tensor("mt", [s, s], BF16, kind="ExternalInput").ap()
    out = nc.dram_tensor("out", [B * s, d], BF16, kind="ExternalOutput").ap()

    with tile.TileContext(nc) as tc:
        with ExitStack() as ctx:
            const = ctx.enter_context(tc.tile_pool(name="const", bufs=1))
            xpool = ctx.enter_context(tc.tile_pool(name="xpool", bufs=2))
            wopool = ctx.enter_context(tc.tile_pool(name="wopool", bufs=2))
            qkv = ctx.enter_context(tc.tile_pool(name="qkv", bufs=2))
            ptp = ctx.enter_context(tc.tile_pool(name="ptp", bufs=2))
            rp = ctx.enter_context(tc.tile_pool(name="rp", bufs=3))
            small = ctx.enter_context(tc.tile_pool(name="small", bufs=2))
            oev = ctx.enter_context(tc.tile_pool(name="oev", bufs=2))
            psm = ctx.enter_context(tc.tile_pool(name="psm", bufs=4, space="PSUM"))
            pss = ctx.enter_context(tc.tile_pool(name="pss", bufs=2, space="PSUM"))

            # resident constants / weights — spread the startup loads over
            # several engines' DMA queues so they transfer in parallel and
            # the first projection matmuls can start early
            # queue order matters: each queue drains in emission order, so
            # load what the first projections need (wk, wq0..) ahead of the
            # rope/exp tables used a few microseconds later
            wk_sb = const.tile([128, dc, HD], BF16)
            nc.gpsimd.dma_start(wk_sb[:], wk[:])
            wq_sb = const.tile([128, NH, dc, HD], BF16)
            nc.scalar.dma_start(wq_sb[:, 0, :, :], wq[:, 0, :, :])
            wv_sb = const.tile([128, dc, HD], BF16)
            nc.gpsimd.dma_start(wv_sb[:], wv[:])
            nc.gpsimd.dma_start(wq_sb[:, 1, :, :], wq[:, 1, :, :])
            nc.scalar.dma_start(wq_sb[:, 2, :, :], wq[:, 2, :, :])
            nc.gpsimd.dma_start(wq_sb[:, 3, :, :], wq[:, 3, :, :])
            c2_sb = const.tile([128, s], F32)
            nc.scalar.dma_start(c2_sb[:], c2[:])
            s2_sb = const.tile([128, s], F32)
            nc.scalar.dma_start(s2_sb[:], s2[:])
            if causal:
                mt_sb = const.tile([128, 128], F32)
                nc.scalar.dma_start(mt_sb[:], mt[:])
            else:
                mt_sb = const.tile([128, nkb, s], BF16)
                nc.scalar.dma_start(mt_sb[:], mt.rearrange("(kb p) q -> p kb q", p=128))
            ones_sb = const.tile([128, 1], BF16)
            nc.vector.memset(ones_sb[:], 1.0)
            id_sb = const.tile([128, 128], BF16)
            nc.gpsimd.dma_start(id_sb[:], sw[:])

            def rope(ps, tok0, w, out_slice):
                """ps: [128, w] psum with raw projected Q/K block (d-permuted).
                out = raw*c2 + halfswap(raw)*s2, written as bf16 to out_slice.
                Only the ACT eviction touches PSUM; the swap is two SBUF
                partition-block DMAs and the muls run from SBUF on gpsimd/DVE."""
                raw = rp.tile([128, w], BF16, tag="raw", name=f"raw_{tok0}")
                nc.scalar.copy(raw[:], ps[:, :w])
                swt = rp.tile([128, w], BF16, tag="swt", name=f"swt_{tok0}")
                nc.sync.dma_start(swt[0:64, :], raw[64:128, :])
                nc.sync.dma_start(swt[64:128, :], raw[0:64, :])
                t1 = rp.tile([128, w], F32, tag="t1", name=f"t1_{tok0}")
                nc.vector.tensor_mul(t1[:], swt[:], s2_sb[:, tok0 : tok0 + w])
                t2 = rp.tile([128, w], F32, tag="t2", name=f"t2_{tok0}")
                nc.vector.tensor_mul(t2[:], raw[:], c2_sb[:, tok0 : tok0 + w])
                nc.gpsimd.tensor_add(out_slice, t2[:], t1[:])

            def phase2(b):
                """Stream xT, project Q/K/V for batch b. Returns the
                per-batch activation tiles."""
                qT_sb = qkv.tile([128, NH, s], BF16, tag="qT", name=f"qT_{b}")
                kT_sb = qkv.tile([128, s], BF16, tag="kT", name=f"kT_{b}")
                vT_sb = qkv.tile([128, s], BF16, tag="vT", name=f"vT_{b}")
                v_sb = qkv.tile([128, nkb, HD], BF16, tag="v", name=f"v_{b}")
                attnT_sb = qkv.tile([128, NH, s], BF16, tag="attnT", name=f"attnT_{b}")

                # evictions/rope are emitted one projection late, so each
                # engine's FIFO only sees work whose PSUM inputs are (nearly)
                # ready — avoids head-of-line blocking behind matmul chains.
                pending = []

                def flush(keep):
                    while len(pending) > keep:
                        kind, ps, tok0_ = pending.pop(0)
                        if kind == "k":
                            rope(ps, tok0_, tb, kT_sb[:, tok0_ : tok0_ + tb])
                        elif kind.startswith("q"):
                            h = int(kind[1:])
                            rope(ps, tok0_, tb, qT_sb[:, h, tok0_ : tok0_ + tb])
                        else:  # vt
                            nc.vector.tensor_copy(vT_sb[:, tok0_ : tok0_ + tb], ps[:])
                            for m2 in range(tb // 128):
                                kb = tok0_ // 128 + m2
                                vtp = pss.tile(
                                    [128, HD], BF16, tag="sm", name=f"vtp_{b}_{kb}"
                                )
                                nc.tensor.transpose(
                                    vtp[:], vT_sb[:, kb * 128 : (kb + 1) * 128], id_sb[:]
                                )
                                nc.scalar.copy(v_sb[:, kb, :], vtp[:])

                xcs = {}

                def mk_k(t4, tok0):
                    def u():
                        xc = xpool.tile([128, dc, tb], BF16, tag="xc", name=f"xc_{b}_{t4}")
                        nc.sync.dma_start(xc[:], xT[:, b * ntc + t4, :, :])
                        xcs[t4] = xc
                        k_ps = psm.tile([128, tb], F32, tag="mm", name=f"kps_{b}_{t4}")
                        for c in range(dc):
                            nc.tensor.matmul(
                                k_ps[:],
                                wk_sb[:, c, :],
                                xcs[t4][:, c, :],
                                start=(c == 0),
                                stop=(c == dc - 1),
                            )
                        pending.append(("k", k_ps, tok0))
                        flush(1)
                    return u

                def mk_vt(t4, tok0):
                    def u():
                        # V projection, transposed like K (wide-N matmuls),
                        # then PE-transposed back to natural [tok, d] layout
                        vt_ps = psm.tile([128, tb], F32, tag="mm", name=f"vtps_{b}_{t4}")
                        for c in range(dc):
                            nc.tensor.matmul(
                                vt_ps[:],
                                wv_sb[:, c, :],
                                xcs[t4][:, c, :],
                                start=(c == 0),
                                stop=(c == dc - 1),
                            )
                        pending.append(("vt", vt_ps, tok0))
                        flush(1)
                    return u

                def mk_q(t4, tok0, h):
                    def u():
                        q_ps = psm.tile([128, tb], F32, tag="mm", name=f"qps_{b}_{t4}_{h}")
                        for c in range(dc):
                            nc.tensor.matmul(
                                q_ps[:],
                                wq_sb[:, h, c, :],
                                xcs[t4][:, c, :],
                                start=(c == 0),
                                stop=(c == dc - 1),
                            )
                        pending.append((f"q{h}", q_ps, tok0))
                        flush(1)
                    return u

                units = []
                for t4 in range(ntc):
                    tok0 = t4 * tb
                    units.append(mk_k(t4, tok0))
                    units.append(mk_vt(t4, tok0))
                    for h in range(NH):
                        units.append(mk_q(t4, tok0, h))
                units.append(lambda: flush(0))
                T = dict(qT=qT_sb, kT=kT_sb, vT=vT_sb, v=v_sb, attnT=attnT_sb)
                return T, units

            def attn(b, T):
                """Attention units for batch b, software-pipelined: PV of head
                h-1 is emitted after the scores/sums of head h, so the softmax-
                denominator chain of head h-1 hides under head h's PE work."""
                qT_sb, kT_sb, v_sb, attnT_sb = T["qT"], T["kT"], T["v"], T["attnT"]
                stage1 = {}

                def attn_scores(h):
                    pT = ptp.tile([128, pt_len], BF16, tag="pt", name=f"pt_{b}_{h}")
                    sums = pss.tile([1, s], F32, tag="sums", bufs=1, name=f"sums_{b}_{h}")
                    for ki in range(nkb):
                        q0 = q0s[ki]
                        for qs_, nq in _chunks(q0, s):
                            sc = psm.tile([128, 512], F32, tag="mm", name=f"sc_{b}_{h}_{ki}_{qs_}")
                            nc.tensor.matmul(
                                sc[:, :nq],
                                kT_sb[:, ki * 128 : (ki + 1) * 128],
                                qT_sb[:, h, qs_ : qs_ + nq],
                                start=True,
                                stop=True,
                            )
                            if causal:
                                if qs_ == q0:  # diagonal block
                                    nc.vector.tensor_add(
                                        sc[:, 0:128], sc[:, 0:128], mt_sb[:]
                                    )
                            else:
                                nc.vector.tensor_add(
                                    sc[:, :nq], sc[:, :nq], mt_sb[:, ki, qs_ : qs_ + nq]
                                )
                            po = offs[ki] + qs_ - q0
                            nc.scalar.activation(
                                pT[:, po : po + nq],
                                sc[:, :nq],
                                mybir.ActivationFunctionType.Exp,
                                scale=SQ,
                            )
                            # denominators accumulate in PSUM across ki; the
                            # causal q-ranges nest, so ki==0 (full range)
                            # starts the group for every column.
                            nc.tensor.matmul(
                                sums[0:1, qs_ : qs_ + nq],
                                ones_sb[:],
                                pT[:, po : po + nq],
                                start=(ki == 0),
                                stop=(ki == nkb - 1),
                                skip_group_check=True,
                            )
                    # denominator chain, split into <=512 column pieces so each
                    # serial stage is short and pieces pipeline across engines
                    nhalf = (s + 511) // 512
                    width = s // nhalf
                    rbrs = []
                    for hs in range(nhalf):
                        ssb = small.tile([1, width], F32, tag="ssb", bufs=4, name=f"ssb_{b}_{h}_{hs}")
                        nc.scalar.copy(ssb[0:1, :], sums[0:1, hs * width : (hs + 1) * width])
                        rb = small.tile([128, width], F32, tag="rb", bufs=4, name=f"rb_{b}_{h}_{hs}")
                        nc.gpsimd.partition_broadcast(rb[:], ssb[0:1, :])
                        rbr = small.tile([128, width], F32, tag="rbr", bufs=4, name=f"rbr_{b}_{h}_{hs}")
                        nc.vector.reciprocal_approx_fast(rbr[:], rb[:])
                        rbrs.append(rbr)
                    return pT, rbrs, width

                def attn_pv(h):
                    # wide-N PV: per q-chunk, each k-block contributes one
                    # matmul over its (nested) valid q range, accumulating in
                    # PSUM — ki==0 always covers the whole chunk, so it opens
                    # the group for every column (same trick as the sums).
                    pT, rbrs, width = stage1.pop(h)
                    for ci, (qs0, w) in enumerate(_chunks(0, s)):
                        o_ps = psm.tile([128, 512], F32, tag="mm", name=f"ops_{b}_{h}_{ci}")
                        kis = [
                            k for k in range(nkb) if (not causal) or q0s[k] < qs0 + w
                        ]
                        for j, ki in enumerate(kis):
                            qlo = max(q0s[ki], qs0)
                            nc.tensor.matmul(
                                o_ps[:, qlo - qs0 : w],
                                v_sb[:, ki, :],
                                pT[:, offs[ki] + qlo - q0s[ki] : offs[ki] + qs0 + w - q0s[ki]],
                                start=(j == 0),
                                stop=(j == len(kis) - 1),
                                skip_group_check=True,
                            )
                        nc.vector.tensor_mul(
                            attnT_sb[:, h, qs0 : qs0 + w],
                            o_ps[:, :w],
                            rbrs[qs0 // width][:, qs0 % width : qs0 % width + w],
                        )

                units = []
                for h in range(NH):
                    units.append(lambda h=h: stage1.__setitem__(h, attn_scores(h)))
                    if h > 0:
                        units.append(lambda h=h: attn_pv(h - 1))
                units.append(lambda: attn_pv(NH - 1))
                return units

            def oproj(b, T):
                """Output projection units (partial over this core's wo rows)."""
                attnT_sb = T["attnT"]
                wo_nbs = {}

                def mk(nb, tp):
                    def u():
                        if tp == 0:
                            wo_nb = wopool.tile(
                                [128, NH, 512], BF16, tag="wo", name=f"wo_{b}_{nb}"
                            )
                            nc.sync.dma_start(wo_nb[:], wo[:, nb, :, :])
                            wo_nbs[nb] = wo_nb
                        ot = oev.tile([128, 2, 512], BF16, tag="ot", bufs=4, name=f"ot_{b}_{nb}_{tp}")
                        for half in range(2):
                            tbk = tp * 2 + half
                            o2 = psm.tile([128, 512], F32, tag="mm", name=f"o2_{b}_{nb}_{tbk}")
                            for h in range(NH):
                                nc.tensor.matmul(
                                    o2[:],
                                    attnT_sb[:, h, tbk * 128 : (tbk + 1) * 128],
                                    wo_nbs[nb][:, h, :],
                                    start=(h == 0),
                                    stop=(h == NH - 1),
                                )
                            if half == 0:
                                nc.scalar.copy(ot[:, half, :], o2[:])
                            else:
                                nc.vector.tensor_copy(ot[:, half, :], o2[:])
                        (nc.sync if tp % 2 == 0 else nc.gpsimd).dma_start(
                            out[
                                b * s + tp * 256 : b * s + (tp + 1) * 256,
                                nb * 512 : (nb + 1) * 512,
                            ].rearrange("(rh p) n -> p rh n", p=128),
                            ot[:],
                        )
                    return u

                return [mk(nb, tp) for nb in range(d // 512) for tp in range(nkb // 2)]

            def zip_emit(primary, filler):
                """Emit primary units with filler units woven between them, so
                the in-order PE queue always has dense matmul work to run
                while the primary's cross-engine chains (exp/softmax) drain."""
                k = max(1, len(filler) // max(1, len(primary)))
                fi = 0
                for u in primary:
                    u()
                    for _ in range(k):
                        if fi < len(filler):
                            filler[fi]()
                            fi += 1
                while fi < len(filler):
                    filler[fi]()
                    fi += 1

            # batch 0 projections run alone; batch 0 attention is interleaved
            # with batch 1 projections; batch 1 attention with batch 0 output
            # projection; batch 1 output projection drains at the end.
            T0, p20 = phase2(0)
            for u in p20:
                u()
            a0 = attn(0, T0)
            if B > 1:
                T1, p21 = phase2(1)
                zip_emit(a0, p21)
                a1 = attn(1, T1)
                o0 = oproj(0, T0)
                zip_emit(a1, o0)
                for u in oproj(1, T1):
                    u()
            else:
                for u in a0:
                    u()
                for u in oproj(0, T0):
                    u()
    nc.compile()
    return nc


# ---------------------------------------------------------------------------
# host side
# ---------------------------------------------------------------------------

_PERM = np.concatenate([np.arange(0, HD, 2), np.arange(1, HD, 2)])
_CACHE = {}


def _tile_x(x, s=S, d=D, tb=TB):
    """[B, s, d] f32 -> [128, B*ntc, dc, tb] bf16 (SBUF layout of xT chunks)."""
    ntc, dc = s // tb, d // 128
    t = x.reshape(B, ntc, tb, dc, 128).transpose(4, 0, 1, 3, 2)
    return np.ascontiguousarray(t.reshape(128, B * ntc, dc, tb)).astype(NPBF16)


def _tile_wq(w, d=D):
    """[d, NH*HD] f32 (already rope-permuted) -> [128, NH, dc, HD] bf16."""
    dc = d // 128
    t = w.reshape(dc, 128, NH, HD).transpose(1, 2, 0, 3)
    return np.ascontiguousarray(t).astype(NPBF16)


def _tile_wkv(w, d=D):
    """[d, HD] f32 -> [128, dc, HD] bf16."""
    dc = d // 128
    return np.ascontiguousarray(w.reshape(dc, 128, HD).transpose(1, 0, 2)).astype(NPBF16)


def _tile_wo(w, d=D):
    """[NH*HD, d] f32 -> [128, d//512, NH, 512] bf16."""
    t = w.reshape(NH, 128, d // 512, 512).transpose(1, 2, 0, 3)
    return np.ascontiguousarray(t).astype(NPBF16)


def _get_program(causal):
    if causal not in _CACHE:
        _CACHE[causal] = build_program(causal)
    return _CACHE[causal]


def _is_causal(mask):
    iu = np.triu_indices(S, 1)
    il = np.tril_indices(S)
    return bool(np.all(mask[il] == 0.0) and np.all(mask[iu] < -1e8))


def make_in_maps(x, cos, sin, mask, wq, wk, wv, wo, causal):
    x = np.asarray(x, dtype=np.float32)
    cos = np.asarray(cos, dtype=np.float32)
    sin = np.asarray(sin, dtype=np.float32)
    mask = np.asarray(mask, dtype=np.float32)
    wq = np.asarray(wq, dtype=np.float32)
    wk = np.asarray(wk, dtype=np.float32)
    wv = np.asarray(wv, dtype=np.float32)
    wo = np.asarray(wo, dtype=np.float32)

    xT = _tile_x(x)
    c2 = np.ascontiguousarray(np.concatenate([cos.T, cos.T], 0)).astype(np.float32)
    s2 = np.ascontiguousarray(np.concatenate([-sin.T, sin.T], 0)).astype(np.float32)
    swm = np.eye(128, dtype=np.float32).astype(NPBF16)  # transpose identity
    if causal:
        mt = np.ascontiguousarray(mask[:128, :128].T * math.sqrt(HD)).astype(np.float32)
    else:
        mt = np.ascontiguousarray(mask.T * math.sqrt(HD)).astype(NPBF16)

    in_maps = []
    for c in range(NCORES):
        wq_c = wq[:, c * NH * HD : (c + 1) * NH * HD].reshape(D, NH, HD)[:, :, _PERM]
        wq_c = _tile_wq(wq_c.reshape(D, NH * HD))
        wk_c = _tile_wkv(np.ascontiguousarray(wk[:, c * HD : (c + 1) * HD][:, _PERM]))
        wv_c = _tile_wkv(np.ascontiguousarray(wv[:, c * HD : (c + 1) * HD]))
        wo_c = _tile_wo(np.ascontiguousarray(wo[c * NH * HD : (c + 1) * NH * HD, :]))
        in_maps.append(
            {
                "xT": xT,
                "wq": wq_c,
                "wk": wk_c,
                "wv": wv_c,
                "wo": wo_c,
                "sw": swm,
                "c2": c2,
                "s2": s2,
                "mt": mt,
            }
        )
    return in_maps


def run(in_maps, causal, **kwargs):
    nc = _get_program(causal)
    return run_bass_kernel_spmd(nc, in_maps, core_ids=list(range(NCORES)), **kwargs)


def kernel(x, start_pos, cos, sin, mask, wq, wk, wv, wo):
    mask = np.asarray(mask, dtype=np.float32)
    causal = _is_causal(mask)
    in_maps = make_in_maps(x, cos, sin, mask, wq, wk, wv, wo, causal)
    res = run(in_maps, causal)
    acc = np.zeros((B * S, D), dtype=np.float32)
    for c in range(NCORES):
        acc += np.asarray(res.results[c]["out"], dtype=np.float32)
    return acc.reshape(B, S, D)
